# revision 22
# baseline (speedup 1.0000x reference)
"""Distributed masked-attention kernel for 8 TRN2 NeuronCores.

Problem: single-head attention, N=4 batches, S=4096, E=512 (f32), with an
elementwise int32 0/1 mask on the [S, S] score matrix.

Sharding: 8 shards = (batch b, query-half h); each core handles 2048 queries
of one batch against all 4096 keys of that batch. Fully data-parallel, no
collectives.

Everything on device runs in the "transposed" domain so the TensorEngine
never needs an on-chip transpose, and both weight products are folded:
  - scoresT[j, i] = kT.T @ q~T, where q~ = q (Wq'.T Wk) folds BOTH
    projection weights into a single [512,512] host-side matrix, so the
    raw (transposed) keys feed the score matmul directly.
  - attnT[j, i]   = exp(scoresT + maskT_bias)  (mask folded in as an
    additive -1e20 bias, pre-transposed on host)
  - out[i, f]     = attnT.T @ v2, where v2 = V (Wo Wv).T folds the output
    projection into the value projection. A ones-column appended to v2
    produces the softmax denominator in the same matmuls.

All compute is bf16 on the TensorEngine (fp8 was tested and rejected:
attention-weight quantization error propagates to the output at full
per-element magnitude). DRAM inputs are laid out host-side so each DMA
descriptor moves 8KB-contiguous runs per partition.
"""

import sys

import numpy as np

if "/opt/trn_rl_repo" not in sys.path:
    sys.path.insert(0, "/opt/trn_rl_repo")

import concourse.bass as bass
import concourse.tile as tile
from concourse import mybir
from concourse.bass_utils import run_bass_kernel_spmd

F32 = mybir.dt.float32
BF16 = mybir.dt.bfloat16

N, S, E = 4, 4096, 512
P = 128
QH = S // 2          # queries per core
ED = E // P          # 4 chunks of the embedding dim
JT = S // P          # 32 key tiles
NQ = 4               # i-quarters per core
IQW = QH // NQ       # 512 queries per quarter
IC = IQW // P        # 4 i-chunks per quarter
KSPAN = 512          # j-span for streaming k/v/q through the prologue
NCORES = 8

NEG_BIG = np.float32(-1e20)


def build_bass():
    nc = bass.Bass()

    # all layouts are pre-tiled on host: [span/group, 128, chunk, width]
    qT = nc.declare_dram_parameter("qT", [QH // KSPAN, P, ED, KSPAN], F32, isOutput=False)
    kT = nc.declare_dram_parameter("kT", [S // KSPAN, P, ED, KSPAN], F32, isOutput=False)
    vT = nc.declare_dram_parameter("vT", [S // KSPAN, P, ED, KSPAN], F32, isOutput=False)
    maskT = nc.declare_dram_parameter("maskT", [NQ, 8, P, 4, IQW], F32, isOutput=False)
    wqk = nc.declare_dram_parameter("wqk", [P, ED, E], F32, isOutput=False)
    w2T = nc.declare_dram_parameter("w2T", [P, ED, E], F32, isOutput=False)
    bo = nc.declare_dram_parameter("bo", [P, E], F32, isOutput=False)
    out = nc.declare_dram_parameter("out", [QH, E], F32, isOutput=True)

    with tile.TileContext(nc) as tc:
        with (
            tc.tile_pool(name="persist", bufs=1) as persist,
            tc.tile_pool(name="xload", bufs=3) as xload,
            tc.tile_pool(name="maskp", bufs=3) as maskp,
            tc.tile_pool(name="smtmp", bufs=3) as smtmp,
            tc.tile_pool(name="attnp", bufs=33) as attnp,
            tc.tile_pool(name="outp", bufs=2) as outp,
            tc.tile_pool(name="small", bufs=8) as small,
            tc.tile_pool(name="ps_pro", bufs=2, space="PSUM") as ps_pro,
            tc.tile_pool(name="ps_s", bufs=2, space="PSUM") as ps_s,
            tc.tile_pool(name="ps_o", bufs=2, space="PSUM") as ps_o,
        ):
            # ---------------- prologue: weights ----------------
            def load_weight_bf16(view, nm):
                f = xload.tile([P, ED, E], F32, tag="xf")
                nc.sync.dma_start(out=f, in_=view[:, :, :])
                b = persist.tile([P, ED, E], BF16, tag=f"wb_{nm}")
                nc.vector.tensor_copy(out=b, in_=f)
                return b

            wqk_b = load_weight_bf16(wqk, "qk")

            # persistent tensors (bf16)
            kb_sb = persist.tile([P, ED, S], BF16)      # raw kT (cast only)
            qp_sb = persist.tile([P, ED, QH], BF16)     # q~T  [d, i]
            v2a = persist.tile([P, JT, 257], BF16)      # v2[:, 0:256] + ones col
            v2b = persist.tile([P, JT, 256], BF16)      # v2[:, 256:512]
            nc.vector.memset(v2a[:, :, 256:257], 1.0)   # ones column only

            def emit_qproj(qs):
                qf = xload.tile([P, ED, KSPAN], F32, tag="xf")
                nc.sync.dma_start(out=qf, in_=qT[qs])
                qb = xload.tile([P, ED, KSPAN], BF16, tag="xb")
                nc.vector.tensor_copy(out=qb, in_=qf)
                for ec in range(ED):
                    ps = ps_pro.tile([P, KSPAN], F32)
                    for dc in range(ED):
                        nc.tensor.matmul(
                            out=ps,
                            lhsT=wqk_b[:, dc, ec * P:(ec + 1) * P],
                            rhs=qb[:, dc, :],
                            start=(dc == 0),
                            stop=(dc == ED - 1),
                        )
                    nc.scalar.copy(
                        out=qp_sb[:, ec, qs * KSPAN:(qs + 1) * KSPAN], in_=ps
                    )

            # quarter 0 only needs Q span 0 projected
            emit_qproj(0)

            # -------- prologue: K cast, interleaved with quarter-0 phase A --
            def emit_strip(q, jt, mtiles, at_tiles):
                ps = ps_s.tile([P, IQW], F32, tag="ps_s")
                for dc in range(ED):
                    nc.tensor.matmul(
                        out=ps,
                        lhsT=kb_sb[:, dc, jt * P:(jt + 1) * P],
                        rhs=qp_sb[:, dc, q * IQW:(q + 1) * IQW],
                        start=(dc == 0),
                        stop=(dc == ED - 1),
                    )
                sm = smtmp.tile([P, IQW], F32, tag="sm")
                nc.vector.tensor_add(
                    out=sm, in0=ps, in1=mtiles[jt // 4][:, jt % 4, :]
                )
                at = attnp.tile([P, IQW], BF16, tag="at")
                nc.scalar.activation(
                    out=at, in_=sm, func=mybir.ActivationFunctionType.Exp
                )
                at_tiles.append(at)

            def emit_vspan(js, w2_b):
                vf = xload.tile([P, ED, KSPAN], F32, tag="xf")
                nc.sync.dma_start(out=vf, in_=vT[js])
                vb = xload.tile([P, ED, KSPAN], BF16, tag="xb")
                nc.vector.tensor_copy(out=vb, in_=vf)
                for jc in range(KSPAN // P):
                    jt = js * (KSPAN // P) + jc
                    ps = ps_pro.tile([P, KSPAN], F32)
                    for dc in range(ED):
                        nc.tensor.matmul(
                            out=ps,
                            lhsT=vb[:, dc, jc * P:(jc + 1) * P],
                            rhs=w2_b[:, dc, :],
                            start=(dc == 0),
                            stop=(dc == ED - 1),
                        )
                    nc.scalar.copy(out=v2a[:, jt, 0:256], in_=ps[:, 0:256])
                    nc.scalar.copy(out=v2b[:, jt, :], in_=ps[:, 256:512])

            def emit_kspan(js):
                kf = xload.tile([P, ED, KSPAN], F32, tag="xf")
                nc.sync.dma_start(out=kf, in_=kT[js])
                nc.vector.tensor_copy(
                    out=kb_sb[:, :, js * KSPAN:(js + 1) * KSPAN], in_=kf
                )

            def b_mms(jt, at, po, ics):
                for ic in ics:
                    for fh, v2t in ((0, v2a), (1, v2b)):
                        width = 257 if fh == 0 else 256
                        nc.tensor.matmul(
                            out=po[(ic, fh)],
                            lhsT=at[:, ic * P:(ic + 1) * P],
                            rhs=v2t[:, jt, 0:width],
                            start=(jt == 0),
                            stop=(jt == JT - 1),
                        )

            def drain_half(q, ics, po, bo_sb):
                for ic in ics:
                    out_sb = outp.tile([P, E], F32, tag="out")
                    r = small.tile([P, 1], F32, tag="r")
                    nc.vector.reciprocal(out=r, in_=po[(ic, 0)][:, 256:257])
                    nc.scalar.copy(out=out_sb[:, 0:256], in_=po[(ic, 0)][:, 0:256])
                    nc.scalar.copy(out=out_sb[:, 256:512], in_=po[(ic, 1)])
                    nc.vector.tensor_scalar_mul(out_sb, out_sb, r)
                    nc.vector.tensor_add(out=out_sb, in0=out_sb, in1=bo_sb)
                    nc.sync.dma_start(
                        out=out[(q * IC + ic) * P:(q * IC + ic + 1) * P, :],
                        in_=out_sb,
                    )

            # ------------- fused main pipeline over query quarters ---------
            for q in range(NQ):
                mtiles = []
                at_tiles = []
                po = {
                    (ic, fh): ps_o.tile(
                        [P, 257 if fh == 0 else 256], F32, tag=f"po{fh}",
                        name=f"po_{q}_{ic}_{fh}",
                    )
                    for ic in (0, 1) for fh in (0, 1)
                }
                for jt in range(JT):
                    js = jt // 4
                    if jt % 4 == 0:
                        if q == 0:
                            emit_kspan(js)
                            if js == 0:
                                w2_b = load_weight_bf16(w2T, "2")
                                bo_sb = persist.tile([P, E], F32)
                                nc.sync.dma_start(out=bo_sb, in_=bo[:, :])
                            emit_vspan(js, w2_b)
                            if js in (2, 4, 6):
                                emit_qproj(js // 2)
                        mt = maskp.tile([P, 4, IQW], F32, tag="mask")
                        nc.sync.dma_start(out=mt, in_=maskT[q, js])
                        mtiles.append(mt)
                    emit_strip(q, jt, mtiles, at_tiles)
                    b_mms(jt, at_tiles[jt], po, (0, 1))
                drain_half(q, (0, 1), po, bo_sb)
                po = {
                    (ic, fh): ps_o.tile(
                        [P, 257 if fh == 0 else 256], F32, tag=f"po{fh}",
                        name=f"po2_{q}_{ic}_{fh}",
                    )
                    for ic in (2, 3) for fh in (0, 1)
                }
                for jt in range(JT):
                    b_mms(jt, at_tiles[jt], po, (2, 3))
                drain_half(q, (2, 3), po, bo_sb)

    _split_waits(nc)
    return nc


def _split_waits(nc):
    """walrus' engine pseudo-instructions accept at most one sync-wait;
    hoist extra waits onto single-wait NoOps on the same engine right
    before the instruction."""
    for f in nc.m.functions:
        for blk in f.blocks:
            new_insts = []
            for inst in blk.instructions:
                si = inst.sync_info
                if si is not None and len(si.on_wait) > 1:
                    waits = list(si.on_wait)
                    for wi, w in enumerate(waits[:-1]):
                        nop = mybir.InstNoOp(
                            name=f"{inst.name}-wsplit{wi}", engine=inst.engine
                        )
                        nop.sync_info = mybir.SyncInfo(on_wait=[w], on_update=[])
                        new_insts.append(nop)
                    inst.sync_info = mybir.SyncInfo(
                        on_wait=waits[-1:], on_update=list(si.on_update)
                    )
                new_insts.append(inst)
            blk.instructions = new_insts


def _tile_rows(a, width):
    """[R(=c*128), M(=s*width)] -> [s, 128, c, width] host relayout so each
    SBUF partition row is one contiguous DRAM run."""
    R, M = a.shape
    c = R // P
    s = M // width
    return np.ascontiguousarray(
        a.reshape(c, P, s, width).transpose(2, 1, 0, 3)
    )


def _prep_core_inputs(values, keys, query, mask, wqk, w2T, bo_rep):
    in_maps = []
    kv_cache = {}
    for c in range(NCORES):
        b, h = divmod(c, 2)
        qs = slice(h * QH, (h + 1) * QH)
        if b not in kv_cache:
            kv_cache[b] = (
                _tile_rows(np.ascontiguousarray(keys[b, 0].T), KSPAN),
                _tile_rows(np.ascontiguousarray(values[b, 0].T), KSPAN),
            )
        kTl, vTl = kv_cache[b]
        qTl = _tile_rows(np.ascontiguousarray(query[b, 0, qs, :].T), KSPAN)
        mbias = np.where(mask[b, 0, qs, :] == 0, NEG_BIG, np.float32(0.0))
        # [j, i] -> [q, g, p, t, i]: j = g*512 + t*128 + p, i = q*512 + iw
        mT = np.ascontiguousarray(
            mbias.T.reshape(8, 4, P, NQ, IQW).transpose(3, 0, 2, 1, 4)
        )
        in_maps.append(
            {
                "qT": qTl,
                "kT": kTl,
                "vT": vTl,
                "maskT": mT,
                "wqk": wqk,
                "w2T": w2T,
                "bo": bo_rep,
            }
        )
    return in_maps


def kernel(values, keys, query, mask, Wv, Wk, Wq, Wo, bo, _profile=False):
    values = np.asarray(values, dtype=np.float32)
    keys = np.asarray(keys, dtype=np.float32)
    query = np.asarray(query, dtype=np.float32)
    mask = np.asarray(mask)
    Wv = np.asarray(Wv, dtype=np.float32)
    Wk = np.asarray(Wk, dtype=np.float32)
    Wq = np.asarray(Wq, dtype=np.float32)
    Wo = np.asarray(Wo, dtype=np.float32)
    bo = np.asarray(bo, dtype=np.float32)

    scale = np.float32(1.0 / np.sqrt(E))
    # A = Wq'.T @ Wk: scores = q A k.T;  lhsT layout [d(part), d2(free)]
    wqk_m = _tile_rows(np.ascontiguousarray((Wq * scale).T @ Wk), E)[0]
    w2T = _tile_rows(np.ascontiguousarray((Wo @ Wv).T), E)[0]
    bo_rep = np.ascontiguousarray(np.broadcast_to(bo, (P, E)))

    in_maps = _prep_core_inputs(values, keys, query, mask, wqk_m, w2T, bo_rep)

    nc = build_bass()
    res = run_bass_kernel_spmd(
        nc, in_maps, core_ids=list(range(NCORES)), trace=_profile
    )

    out = np.empty((N, S, E), dtype=np.float32)
    for c in range(NCORES):
        b, h = divmod(c, 2)
        out[b, h * QH:(h + 1) * QH, :] = res.results[c]["out"]

    if _profile:
        return out, res
    return out


if __name__ == "__main__":
    rng = np.random.default_rng(0)
    inputs = {
        "values": rng.standard_normal((N, 1, S, E), dtype=np.float32),
        "keys": rng.standard_normal((N, 1, S, E), dtype=np.float32),
        "query": rng.standard_normal((N, 1, S, E), dtype=np.float32),
        "mask": rng.integers(0, 2, size=(N, 1, S, S)).astype(np.int32),
        "Wv": rng.standard_normal((E, E), dtype=np.float32) / np.sqrt(E),
        "Wk": rng.standard_normal((E, E), dtype=np.float32) / np.sqrt(E),
        "Wq": rng.standard_normal((E, E), dtype=np.float32) / np.sqrt(E),
        "Wo": rng.standard_normal((E, E), dtype=np.float32) / np.sqrt(E),
        "bo": np.zeros((E,), dtype=np.float32),
    }
    out = kernel(**inputs)
    print("out shape:", out.shape, out.dtype)


# revision 24
# speedup vs baseline: 1.0101x; 1.0101x over previous
"""Distributed masked-attention kernel for 8 TRN2 NeuronCores.

Problem: single-head attention, N=4 batches, S=4096, E=512 (f32), with an
elementwise int32 0/1 mask on the [S, S] score matrix.

Sharding: 8 shards = (batch b, query-half h); each core handles 2048 queries
of one batch against all 4096 keys of that batch. Fully data-parallel, no
collectives.

Everything on device runs in the "transposed" domain so the TensorEngine
never needs an on-chip transpose, and both weight products are folded:
  - scoresT[j, i] = kT.T @ q~T, where q~ = q (Wq'.T Wk) folds BOTH
    projection weights into a single [512,512] host-side matrix, so the
    raw (transposed) keys feed the score matmul directly.
  - attnT[j, i]   = exp(scoresT + maskT_bias)  (mask folded in as an
    additive -1e20 bias, pre-transposed on host)
  - out[i, f]     = attnT.T @ v2, where v2 = V (Wo Wv).T folds the output
    projection into the value projection. A ones-column appended to v2
    produces the softmax denominator in the same matmuls.

All compute is bf16 on the TensorEngine (fp8 was tested and rejected:
attention-weight quantization error propagates to the output at full
per-element magnitude). DRAM inputs are laid out host-side so each DMA
descriptor moves 8KB-contiguous runs per partition.
"""

import sys

import numpy as np

if "/opt/trn_rl_repo" not in sys.path:
    sys.path.insert(0, "/opt/trn_rl_repo")

import concourse.bass as bass
import concourse.tile as tile
from concourse import mybir
from concourse.bass_utils import run_bass_kernel_spmd

F32 = mybir.dt.float32
BF16 = mybir.dt.bfloat16

N, S, E = 4, 4096, 512
P = 128
QH = S // 2          # queries per core
ED = E // P          # 4 chunks of the embedding dim
JT = S // P          # 32 key tiles
NQ = 4               # i-quarters per core
IQW = QH // NQ       # 512 queries per quarter
IC = IQW // P        # 4 i-chunks per quarter
KSPAN = 512          # j-span for streaming k/v/q through the prologue
NCORES = 8

NEG_BIG = np.float32(-1e20)


def build_bass():
    nc = bass.Bass()

    # all layouts are pre-tiled on host: [span/group, 128, chunk, width]
    qT = nc.declare_dram_parameter("qT", [QH // KSPAN, P, ED, KSPAN], F32, isOutput=False)
    kT = nc.declare_dram_parameter("kT", [S // KSPAN, P, ED, KSPAN], F32, isOutput=False)
    vT = nc.declare_dram_parameter("vT", [S // KSPAN, P, ED, KSPAN], F32, isOutput=False)
    maskT = nc.declare_dram_parameter("maskT", [NQ, 8, P, 4, IQW], F32, isOutput=False)
    wqk = nc.declare_dram_parameter("wqk", [P, ED, E], F32, isOutput=False)
    w2T = nc.declare_dram_parameter("w2T", [P, ED, E], F32, isOutput=False)
    bo = nc.declare_dram_parameter("bo", [P, E], F32, isOutput=False)
    out = nc.declare_dram_parameter("out", [QH, E], F32, isOutput=True)

    with tile.TileContext(nc) as tc:
        with (
            tc.tile_pool(name="persist", bufs=1) as persist,
            tc.tile_pool(name="xload", bufs=3) as xload,
            tc.tile_pool(name="maskp", bufs=3) as maskp,
            tc.tile_pool(name="smtmp", bufs=3) as smtmp,
            tc.tile_pool(name="attnp", bufs=33) as attnp,
            tc.tile_pool(name="outp", bufs=2) as outp,
            tc.tile_pool(name="small", bufs=8) as small,
            tc.tile_pool(name="ps_pro", bufs=2, space="PSUM") as ps_pro,
            tc.tile_pool(name="ps_s", bufs=2, space="PSUM") as ps_s,
            tc.tile_pool(name="ps_o", bufs=2, space="PSUM") as ps_o,
        ):
            # warm the PE clock gate with tiny const matmuls so the first
            # real matmuls run at 2.4GHz instead of 1.2GHz
            warm_ps = ps_pro.tile([1, 1], F32, name="warm_ps", tag="ps")
            cap = nc.const_aps.tensor(1.0, (P, 1), BF16)
            for _ in range(150):
                nc.tensor.matmul(out=warm_ps, lhsT=cap, rhs=cap,
                                 start=True, stop=True)

            # ---------------- prologue: weights ----------------
            def load_weight_bf16(view, nm):
                f = xload.tile([P, ED, E], F32, tag="xf")
                nc.sync.dma_start(out=f, in_=view[:, :, :])
                b = persist.tile([P, ED, E], BF16, tag=f"wb_{nm}")
                nc.vector.tensor_copy(out=b, in_=f)
                return b

            wqk_b = load_weight_bf16(wqk, "qk")

            # persistent tensors (bf16)
            kb_sb = persist.tile([P, ED, S], BF16)      # raw kT (cast only)
            qp_sb = persist.tile([P, ED, QH], BF16)     # q~T  [d, i]
            v2a = persist.tile([P, JT, 257], BF16)      # v2[:, 0:256] + ones col
            v2b = persist.tile([P, JT, 256], BF16)      # v2[:, 256:512]
            nc.vector.memset(v2a[:, :, 256:257], 1.0)   # ones column only

            def emit_qproj(qs):
                qf = xload.tile([P, ED, KSPAN], F32, tag="xf")
                nc.sync.dma_start(out=qf, in_=qT[qs])
                qb = xload.tile([P, ED, KSPAN], BF16, tag="xb")
                nc.vector.tensor_copy(out=qb, in_=qf)
                for ec in range(ED):
                    ps = ps_pro.tile([P, KSPAN], F32)
                    for dc in range(ED):
                        nc.tensor.matmul(
                            out=ps,
                            lhsT=wqk_b[:, dc, ec * P:(ec + 1) * P],
                            rhs=qb[:, dc, :],
                            start=(dc == 0),
                            stop=(dc == ED - 1),
                        )
                    nc.scalar.copy(
                        out=qp_sb[:, ec, qs * KSPAN:(qs + 1) * KSPAN], in_=ps
                    )

            # quarter 0 only needs Q span 0 projected
            emit_qproj(0)

            # -------- prologue: K cast, interleaved with quarter-0 phase A --
            def emit_strip(q, jt, mtiles, at_tiles):
                ps = ps_s.tile([P, IQW], F32, tag="ps_s")
                for dc in range(ED):
                    nc.tensor.matmul(
                        out=ps,
                        lhsT=kb_sb[:, dc, jt * P:(jt + 1) * P],
                        rhs=qp_sb[:, dc, q * IQW:(q + 1) * IQW],
                        start=(dc == 0),
                        stop=(dc == ED - 1),
                    )
                sm = smtmp.tile([P, IQW], F32, tag="sm")
                nc.vector.tensor_add(
                    out=sm, in0=ps, in1=mtiles[jt // 4][:, jt % 4, :]
                )
                at = attnp.tile([P, IQW], BF16, tag="at")
                nc.scalar.activation(
                    out=at, in_=sm, func=mybir.ActivationFunctionType.Exp
                )
                at_tiles.append(at)

            def emit_vspan(js, w2_b):
                vf = xload.tile([P, ED, KSPAN], F32, tag="xf")
                nc.sync.dma_start(out=vf, in_=vT[js])
                vb = xload.tile([P, ED, KSPAN], BF16, tag="xb")
                nc.vector.tensor_copy(out=vb, in_=vf)
                for jc in range(KSPAN // P):
                    jt = js * (KSPAN // P) + jc
                    ps = ps_pro.tile([P, KSPAN], F32)
                    for dc in range(ED):
                        nc.tensor.matmul(
                            out=ps,
                            lhsT=vb[:, dc, jc * P:(jc + 1) * P],
                            rhs=w2_b[:, dc, :],
                            start=(dc == 0),
                            stop=(dc == ED - 1),
                        )
                    nc.scalar.copy(out=v2a[:, jt, 0:256], in_=ps[:, 0:256])
                    nc.scalar.copy(out=v2b[:, jt, :], in_=ps[:, 256:512])

            def emit_kspan(js):
                kf = xload.tile([P, ED, KSPAN], F32, tag="xf")
                nc.sync.dma_start(out=kf, in_=kT[js])
                nc.vector.tensor_copy(
                    out=kb_sb[:, :, js * KSPAN:(js + 1) * KSPAN], in_=kf
                )

            def b_mms(jt, at, po, ics):
                for ic in ics:
                    for fh, v2t in ((0, v2a), (1, v2b)):
                        width = 257 if fh == 0 else 256
                        nc.tensor.matmul(
                            out=po[(ic, fh)],
                            lhsT=at[:, ic * P:(ic + 1) * P],
                            rhs=v2t[:, jt, 0:width],
                            start=(jt == 0),
                            stop=(jt == JT - 1),
                        )

            def drain_half(q, ics, po, bo_sb):
                for ic in ics:
                    out_sb = outp.tile([P, E], F32, tag="out")
                    r = small.tile([P, 1], F32, tag="r")
                    nc.vector.reciprocal(out=r, in_=po[(ic, 0)][:, 256:257])
                    nc.scalar.copy(out=out_sb[:, 0:256], in_=po[(ic, 0)][:, 0:256])
                    nc.scalar.copy(out=out_sb[:, 256:512], in_=po[(ic, 1)])
                    nc.vector.tensor_scalar_mul(out_sb, out_sb, r)
                    nc.vector.tensor_add(out=out_sb, in0=out_sb, in1=bo_sb)
                    nc.sync.dma_start(
                        out=out[(q * IC + ic) * P:(q * IC + ic + 1) * P, :],
                        in_=out_sb,
                    )

            # ------------- fused main pipeline over query quarters ---------
            for q in range(NQ):
                mtiles = []
                at_tiles = []
                po = {
                    (ic, fh): ps_o.tile(
                        [P, 257 if fh == 0 else 256], F32, tag=f"po{fh}",
                        name=f"po_{q}_{ic}_{fh}",
                    )
                    for ic in (0, 1) for fh in (0, 1)
                }
                for jt in range(JT):
                    js = jt // 4
                    if jt % 4 == 0:
                        if q == 0:
                            emit_kspan(js)
                            if js == 0:
                                w2_b = load_weight_bf16(w2T, "2")
                                bo_sb = persist.tile([P, E], F32)
                                nc.sync.dma_start(out=bo_sb, in_=bo[:, :])
                            emit_vspan(js, w2_b)
                            if js in (2, 4, 6):
                                emit_qproj(js // 2)
                        mt = maskp.tile([P, 4, IQW], F32, tag="mask")
                        nc.sync.dma_start(out=mt, in_=maskT[q, js])
                        mtiles.append(mt)
                    emit_strip(q, jt, mtiles, at_tiles)
                    b_mms(jt, at_tiles[jt], po, (0, 1))
                drain_half(q, (0, 1), po, bo_sb)
                po = {
                    (ic, fh): ps_o.tile(
                        [P, 257 if fh == 0 else 256], F32, tag=f"po{fh}",
                        name=f"po2_{q}_{ic}_{fh}",
                    )
                    for ic in (2, 3) for fh in (0, 1)
                }
                for jt in range(JT):
                    b_mms(jt, at_tiles[jt], po, (2, 3))
                drain_half(q, (2, 3), po, bo_sb)

    _split_waits(nc)
    return nc


def _split_waits(nc):
    """walrus' engine pseudo-instructions accept at most one sync-wait;
    hoist extra waits onto single-wait NoOps on the same engine right
    before the instruction."""
    for f in nc.m.functions:
        for blk in f.blocks:
            new_insts = []
            for inst in blk.instructions:
                si = inst.sync_info
                if si is not None and len(si.on_wait) > 1:
                    waits = list(si.on_wait)
                    for wi, w in enumerate(waits[:-1]):
                        nop = mybir.InstNoOp(
                            name=f"{inst.name}-wsplit{wi}", engine=inst.engine
                        )
                        nop.sync_info = mybir.SyncInfo(on_wait=[w], on_update=[])
                        new_insts.append(nop)
                    inst.sync_info = mybir.SyncInfo(
                        on_wait=waits[-1:], on_update=list(si.on_update)
                    )
                new_insts.append(inst)
            blk.instructions = new_insts


def _tile_rows(a, width):
    """[R(=c*128), M(=s*width)] -> [s, 128, c, width] host relayout so each
    SBUF partition row is one contiguous DRAM run."""
    R, M = a.shape
    c = R // P
    s = M // width
    return np.ascontiguousarray(
        a.reshape(c, P, s, width).transpose(2, 1, 0, 3)
    )


def _prep_core_inputs(values, keys, query, mask, wqk, w2T, bo_rep):
    in_maps = []
    kv_cache = {}
    for c in range(NCORES):
        b, h = divmod(c, 2)
        qs = slice(h * QH, (h + 1) * QH)
        if b not in kv_cache:
            kv_cache[b] = (
                _tile_rows(np.ascontiguousarray(keys[b, 0].T), KSPAN),
                _tile_rows(np.ascontiguousarray(values[b, 0].T), KSPAN),
            )
        kTl, vTl = kv_cache[b]
        qTl = _tile_rows(np.ascontiguousarray(query[b, 0, qs, :].T), KSPAN)
        mbias = np.where(mask[b, 0, qs, :] == 0, NEG_BIG, np.float32(0.0))
        # [j, i] -> [q, g, p, t, i]: j = g*512 + t*128 + p, i = q*512 + iw
        mT = np.ascontiguousarray(
            mbias.T.reshape(8, 4, P, NQ, IQW).transpose(3, 0, 2, 1, 4)
        )
        in_maps.append(
            {
                "qT": qTl,
                "kT": kTl,
                "vT": vTl,
                "maskT": mT,
                "wqk": wqk,
                "w2T": w2T,
                "bo": bo_rep,
            }
        )
    return in_maps


def kernel(values, keys, query, mask, Wv, Wk, Wq, Wo, bo, _profile=False):
    values = np.asarray(values, dtype=np.float32)
    keys = np.asarray(keys, dtype=np.float32)
    query = np.asarray(query, dtype=np.float32)
    mask = np.asarray(mask)
    Wv = np.asarray(Wv, dtype=np.float32)
    Wk = np.asarray(Wk, dtype=np.float32)
    Wq = np.asarray(Wq, dtype=np.float32)
    Wo = np.asarray(Wo, dtype=np.float32)
    bo = np.asarray(bo, dtype=np.float32)

    scale = np.float32(1.0 / np.sqrt(E))
    # A = Wq'.T @ Wk: scores = q A k.T;  lhsT layout [d(part), d2(free)]
    wqk_m = _tile_rows(np.ascontiguousarray((Wq * scale).T @ Wk), E)[0]
    w2T = _tile_rows(np.ascontiguousarray((Wo @ Wv).T), E)[0]
    bo_rep = np.ascontiguousarray(np.broadcast_to(bo, (P, E)))

    in_maps = _prep_core_inputs(values, keys, query, mask, wqk_m, w2T, bo_rep)

    nc = build_bass()
    res = run_bass_kernel_spmd(
        nc, in_maps, core_ids=list(range(NCORES)), trace=_profile
    )

    out = np.empty((N, S, E), dtype=np.float32)
    for c in range(NCORES):
        b, h = divmod(c, 2)
        out[b, h * QH:(h + 1) * QH, :] = res.results[c]["out"]

    if _profile:
        return out, res
    return out


if __name__ == "__main__":
    rng = np.random.default_rng(0)
    inputs = {
        "values": rng.standard_normal((N, 1, S, E), dtype=np.float32),
        "keys": rng.standard_normal((N, 1, S, E), dtype=np.float32),
        "query": rng.standard_normal((N, 1, S, E), dtype=np.float32),
        "mask": rng.integers(0, 2, size=(N, 1, S, S)).astype(np.int32),
        "Wv": rng.standard_normal((E, E), dtype=np.float32) / np.sqrt(E),
        "Wk": rng.standard_normal((E, E), dtype=np.float32) / np.sqrt(E),
        "Wq": rng.standard_normal((E, E), dtype=np.float32) / np.sqrt(E),
        "Wo": rng.standard_normal((E, E), dtype=np.float32) / np.sqrt(E),
        "bo": np.zeros((E,), dtype=np.float32),
    }
    out = kernel(**inputs)
    print("out shape:", out.shape, out.dtype)
